# revision 11
# baseline (speedup 1.0000x reference)
"""GsplatRGB alpha kernel for 8 Trainium2 NeuronCores.

Math: for each (pose b, gaussian n), alpha[b,y,x,n] = min(op_n * exp(-0.5*prob), 1)
where prob is an exact quadratic in pixel coords (x, y).  All per-gaussian
work (camera transform, projection Jacobian, det) collapses to 6 quadratic
coefficients per (b, n), computed on host in f64 (B*N = 2048 items).

Device work per core (16 of 128 tile rows x 4 poses):
  z[x, n] = basis(x, y)[18] . coef_b[18]  -- one K=18 fp32r matmul per row
  alpha = exp(z)                          -- ScalarE, 4-row batches from PSUM
  DMA out 1MB chunks.

fp32r (1+8+11-bit) runs 4x faster than fp32 on the PE; full fp32 precision is
recovered by an error-compensated split: with B = Br + Bres, C = Cr + Cres
(each part fp32r-exact), z = Br.Cr + Bres.Cr + Br.Cres (+O(2^-24) dropped),
stacked as one K=18 contraction.  Products of two 12-bit significands are
exact in the fp32 PSUM accumulator.

min(alpha, 1) never binds: op <= 0.95 and exp(-0.5*prob) <= 1.
"""
import numpy as np

N_CORES = 8
B, N = 4, 512
H, W = 128, 128
FX, FY = 1000.0, 1000.0
IMG_W, IMG_H = 1024.0, 1024.0
CX, CY = 63.5, 63.5  # basis recentering (reduces cancellation magnitude)
ROWS_PER_CORE = H // N_CORES  # 16
CHUNK = 4  # rows per PSUM/exp/DMA batch

_COMPILED = None


def _rnd_fp32r(a):
    """Round f32 to fp32r (11 explicit mantissa bits), round-to-nearest-even."""
    u = np.asarray(a, np.float32).view(np.uint32).astype(np.uint64)
    keep_lsb = (u >> np.uint64(13)) & np.uint64(1)
    u = (u + np.uint64(0x0FFF) + keep_lsb) & np.uint64(0xFFFFFFFFFFFFE000)
    return u.astype(np.uint32).view(np.float32)


def _host_coefs(pose, means, quats, scales, opacities):
    """coef[B, 6, N] (f64): z = c0 x'^2 + c1 y'^2 + c2 x'y' + c3 x' + c4 y' + c5,
    x' = x - CX, y' = y - CY, such that alpha = exp(z)."""
    dtype = np.float64
    pose = pose.astype(dtype)
    means = means.astype(dtype)
    quats = quats.astype(dtype)
    scales = scales.astype(dtype)
    op = opacities.astype(dtype)[:, 0]
    n = means.shape[0]

    q = quats / np.linalg.norm(quats, axis=-1, keepdims=True)
    w, x, y, z = q[:, 0], q[:, 1], q[:, 2], q[:, 3]
    R = np.stack([
        1 - 2 * (y * y + z * z), 2 * (x * y - w * z), 2 * (x * z + w * y),
        2 * (x * y + w * z), 1 - 2 * (x * x + z * z), 2 * (y * z - w * x),
        2 * (x * z - w * y), 2 * (y * z + w * x), 1 - 2 * (x * x + y * y),
    ], axis=-1).reshape(n, 3, 3)
    Mw = R * scales[:, None, :]

    means_h = np.concatenate([means, np.ones((n, 1), dtype)], axis=1)
    mc = np.einsum('bij,nj->bni', pose, means_h)[:, :, :3]
    us, vs, d = mc[..., 0], mc[..., 1], mc[..., 2]
    Mc = np.einsum('bij,njk->bnik', pose[:, :3, :3], Mw)

    m0 = FX * (d[..., None] * Mc[:, :, 0, :] - us[..., None] * Mc[:, :, 2, :])
    m1 = FY * (d[..., None] * Mc[:, :, 1, :] - vs[..., None] * Mc[:, :, 2, :])

    det = ((m0[..., 0] * m1[..., 1] - m0[..., 1] * m1[..., 0]) ** 2
           + (m0[..., 0] * m1[..., 2] - m0[..., 2] * m1[..., 0]) ** 2
           + (m0[..., 1] * m1[..., 2] - m0[..., 2] * m1[..., 1]) ** 2)

    mpx = FX * us + (IMG_W / 2) * d
    mpy = FY * vs + (IMG_H / 2) * d

    P = d[..., None] ** 2 * m1
    Q = -(d[..., None] ** 2) * m0
    Rk = (mpy * d)[..., None] * m0 - (mpx * d)[..., None] * m1
    Rk = Rk + CX * P + CY * Q  # recentered basis

    s = -0.5 / det
    c_x2 = s * (P * P).sum(-1)
    c_y2 = s * (Q * Q).sum(-1)
    c_xy = 2 * s * (P * Q).sum(-1)
    c_x = 2 * s * (P * Rk).sum(-1)
    c_y = 2 * s * (Q * Rk).sum(-1)
    c_1 = s * (Rk * Rk).sum(-1) + np.log(op)[None, :]
    return np.stack([c_x2, c_y2, c_xy, c_x, c_y, c_1], axis=1)  # [B,6,N]


def _split_fp32r(a32):
    """a32 (f32) -> (hi, lo) both fp32r-exact with hi+lo ~ a32 to ~2^-23."""
    hi = _rnd_fp32r(a32)
    lo = _rnd_fp32r((a32.astype(np.float64) - hi.astype(np.float64)).astype(np.float32))
    return hi, lo


def _build_program():
    import concourse.tile as tile
    from concourse import bacc, mybir

    nc = bacc.Bacc("TRN2", target_bir_lowering=False, debug=False,
                   num_devices=N_CORES)

    # packed params: [basis rows 0-4 | coef_pose0 (N) | basis rows 5.. | coef poses 1..]
    HEAD_ROWS = 5
    NP0 = HEAD_ROWS * W + N
    NPR = (ROWS_PER_CORE - HEAD_ROWS) * W + (B - 1) * N
    params_in = nc.dram_tensor(
        "params", [18, NP0 + NPR], mybir.dt.float32r, kind="ExternalInput").ap()
    out_t = nc.dram_tensor(
        "out", [B, W, ROWS_PER_CORE, N], mybir.dt.float32, kind="ExternalOutput").ap()

    with tile.TileContext(nc) as tc:
        with (
            tc.tile_pool(name="const", bufs=1) as const_pool,
            tc.tile_pool(name="psum", bufs=2, space="PSUM") as psum_pool,
            tc.tile_pool(name="outb", bufs=4) as out_pool,
        ):
            # Two input DMAs: first-chunk data (row0+pose0) in one small
            # transfer so the pipe starts ASAP, then everything else.
            p0_t = const_pool.tile([18, NP0], mybir.dt.float32r, tag="p0")
            nc.sync.dma_start(out=p0_t[:], in_=params_in[:, 0:NP0])
            pr_t = const_pool.tile([18, NPR], mybir.dt.float32r, tag="prest")
            nc.sync.dma_start(out=pr_t[:], in_=params_in[:, NP0:])

            def basis_ap(yl):
                return (p0_t[:, yl * W:(yl + 1) * W] if yl < HEAD_ROWS
                        else pr_t[:, (yl - HEAD_ROWS) * W:(yl - HEAD_ROWS + 1) * W])

            COFF = (ROWS_PER_CORE - HEAD_ROWS) * W

            def coef_ap(b):
                return (p0_t[:, HEAD_ROWS * W:HEAD_ROWS * W + N] if b == 0
                        else pr_t[:, COFF + (b - 1) * N: COFF + b * N])

            # pose 0 starts with a 1-row prologue to warm the pipe; the very
            # last chunk is split 2+2 so the final (serial-tail) DMA is 0.5MB.
            chunks = {0: [(0, 1), (1, 5), (5, 9), (9, 13), (13, 16)]}
            full = [(i, i + CHUNK) for i in range(0, ROWS_PER_CORE, CHUNK)]
            for b in range(1, B):
                chunks[b] = full
            chunks[B - 1] = [(0, 4), (4, 8), (8, 12), (12, 14), (14, 16)]

            for b in range(B):
                for (ys, ye) in chunks[b]:
                    rows = ye - ys
                    ptile = psum_pool.tile([128, CHUNK * N], mybir.dt.float32)
                    for j in range(rows):
                        nc.tensor.matmul(
                            out=ptile[:, j * N:(j + 1) * N],
                            lhsT=basis_ap(ys + j),
                            rhs=coef_ap(b),
                            start=True, stop=True,
                        )
                    otile = out_pool.tile([128, CHUNK * N], mybir.dt.float32)
                    nc.scalar.activation(otile[:, :rows * N], ptile[:, :rows * N],
                                         mybir.ActivationFunctionType.Exp)
                    nc.sync.dma_start(
                        out=out_t[b, :, ys:ye, :],
                        in_=otile[:, :rows * N].rearrange(
                            "p (a c) -> p a c", a=rows),
                    )

    nc.compile()
    return nc


def _get_compiled():
    global _COMPILED
    if _COMPILED is None:
        _COMPILED = _build_program()
    return _COMPILED


def _make_basis(ys):
    """basis rows for given absolute y values -> [18, len(ys)*W] f32 (fp32r split)."""
    xs = np.arange(W, dtype=np.float64) - CX
    ysc = np.asarray(ys, np.float64) - CY
    Xg = np.tile(xs, len(ysc))                      # [R*W]
    Yg = np.repeat(ysc, W)
    B6 = np.stack([Xg * Xg, Yg * Yg, Xg * Yg, Xg, Yg, np.ones_like(Xg)], axis=0)
    B32 = B6.astype(np.float32)
    hi, lo = _split_fp32r(B32)
    return np.concatenate([hi, lo, hi], axis=0)     # [18, R*W]


def _pack_params(basis18, coef18):
    """Pack [18, R*W] basis + [18, B*N] coef into the kernel's params layout:
    [basis rows 0-4 | coef_pose0 | basis rows 5.. | coef poses 1..]."""
    HW_ = 5 * W
    return np.ascontiguousarray(np.concatenate(
        [basis18[:, :HW_], coef18[:, :N], basis18[:, HW_:], coef18[:, N:]],
        axis=1), np.float32)


def kernel(pose, means, quats, scales, opacities):
    from concourse.bass_utils import run_bass_kernel_spmd

    assert pose.shape == (B, 4, 4) and means.shape == (N, 3)
    nc = _get_compiled()

    coef = _host_coefs(pose, means, quats, scales, opacities)  # [B,6,N] f64
    C32 = coef.astype(np.float32)
    Chi, Clo = _split_fp32r(C32)
    # K=18 pairing: lhs [Br; Bres; Br] . rhs [Cr; Cr; Cres]
    coef_np = np.concatenate([Chi, Chi, Clo], axis=1)  # [B,18,N]
    coef_np = coef_np.transpose(1, 0, 2).reshape(18, B * N).copy()  # [18, B*N]
    coef_np = np.ascontiguousarray(coef_np, np.float32)

    in_maps = []
    for c in range(N_CORES):
        ys = np.arange(c * ROWS_PER_CORE, (c + 1) * ROWS_PER_CORE)
        in_maps.append({"params": _pack_params(_make_basis(ys), coef_np)})

    res = run_bass_kernel_spmd(nc, in_maps, list(range(N_CORES)))
    # per-core out: [B, W, ROWS_PER_CORE, N] -> [B, ROWS_PER_CORE, W, N]
    parts = [res.results[c]["out"].transpose(0, 2, 1, 3) for c in range(N_CORES)]
    full = np.concatenate(parts, axis=1)  # [B, H, W, N]
    return np.ascontiguousarray(full[..., None], np.float32)
